# revision 4
# baseline (speedup 1.0000x reference)
"""Bahdanau attention kernel for 8 Trainium2 NeuronCores.

Problem shapes (hardcoded): hidden [2, 32, 1024], encoder_outputs [32, 2048, 1024],
Wq/Wk [1024, 1024], bq/bk/wv [1024], bv scalar. Output [32, 1, 1024].

Sharding: data-parallel over batch B=32 -> 4 batches per core, weights replicated.
bv is dropped entirely (softmax is invariant to constant shifts).
"""

from contextlib import ExitStack

import numpy as np

import concourse.bacc as bacc
import concourse.bass as bass
import concourse.mybir as mybir
import concourse.tile as tile
from concourse.bass_utils import run_bass_kernel_spmd
from concourse.masks import make_identity

B, S, H = 32, 2048, 1024
NCORES = 8
BPC = B // NCORES  # 4 batches per core
F32 = mybir.dt.float32
F32R = mybir.dt.float32r
HT = H // 128  # 8 chunks of 128 along h or o
ST = S // 128  # 16 s-tiles of 128
SC = S // 512  # 4 s-chunks of 512
Tanh = mybir.ActivationFunctionType.Tanh
Exp = mybir.ActivationFunctionType.Exp
X = mybir.AxisListType.X

ts = bass.ts


def r(ap):
    """View an fp32 AP as float32r for full-rate PE matmuls."""
    return ap.bitcast(F32R)


def build_program():
    nc = bacc.Bacc("TRN2", target_bir_lowering=False, debug=False)

    hid_d = nc.dram_tensor("hid", [BPC, H], F32, kind="ExternalInput")
    enc_d = nc.dram_tensor("enc", [BPC, S, H], F32R, kind="ExternalInput")
    wq_d = nc.dram_tensor("wq", [H, H], F32, kind="ExternalInput")
    wk_d = nc.dram_tensor("wk", [H, H], F32, kind="ExternalInput")
    bq_d = nc.dram_tensor("bq", [1, H], F32, kind="ExternalInput")
    bk_d = nc.dram_tensor("bk", [1, H], F32, kind="ExternalInput")
    wv_d = nc.dram_tensor("wv", [1, H], F32, kind="ExternalInput")
    out_d = nc.dram_tensor("out", [BPC, 1, H], F32, kind="ExternalOutput")

    with tile.TileContext(nc) as tc, ExitStack() as ctx:
        consts = ctx.enter_context(tc.tile_pool(name="consts", bufs=1))
        stage = ctx.enter_context(tc.tile_pool(name="stage", bufs=2))
        encnat = ctx.enter_context(tc.tile_pool(name="encnat", bufs=1))
        encT_p = ctx.enter_context(tc.tile_pool(name="encT", bufs=2))
        eT_p = ctx.enter_context(tc.tile_pool(name="eT", bufs=2))
        batch = ctx.enter_context(tc.tile_pool(name="batch", bufs=1))
        tp = ctx.enter_context(tc.tile_pool(name="tp", bufs=2, space="PSUM"))
        kp = ctx.enter_context(tc.tile_pool(name="kp", bufs=3, space="PSUM"))
        vp = ctx.enter_context(tc.tile_pool(name="vp", bufs=2, space="PSUM"))

        ident = consts.tile([128, 128], F32, tag="ident")
        make_identity(nc, ident[:])

        # ---- biases: bsum[o(part), o-chunk] = bq + bk ----
        brow = consts.tile([1, H], F32, tag="brow")
        brow2 = consts.tile([1, H], F32, tag="brow2")
        nc.sync.dma_start(brow[:], bq_d[:])
        nc.sync.dma_start(brow2[:], bk_d[:])
        nc.vector.tensor_add(brow[:], brow[:], brow2[:])
        bsum = consts.tile([128, HT], F32, tag="bsum")
        for c in range(HT):
            pa = tp.tile([128, 1], F32, tag="tp")
            nc.tensor.transpose(pa[:], brow[0:1, ts(c, 128)], ident[0:1, 0:1])
            nc.vector.tensor_copy(bsum[:, c : c + 1], pa[:])

        # ---- wv -> wvT[o(part), o-chunk] ----
        wvrow = consts.tile([1, H], F32, tag="wvrow")
        nc.sync.dma_start(wvrow[:], wv_d[:])
        wvT = consts.tile([128, HT], F32R, tag="wvT")
        for c in range(HT):
            pa = tp.tile([128, 1], F32, tag="tp")
            nc.tensor.transpose(pa[:], wvrow[0:1, ts(c, 128)], ident[0:1, 0:1])
            nc.vector.tensor_copy(wvT[:, c : c + 1], pa[:])

        # ---- hidden slice -> hidT[h(part), h-chunk, b] ----
        hid_nat = consts.tile([BPC, H], F32, tag="hidnat")
        nc.sync.dma_start(hid_nat[:], hid_d[:])
        hidT = consts.tile([128, HT, BPC], F32, tag="hidT")
        for c in range(HT):
            pa = tp.tile([128, BPC], F32, tag="tp")
            nc.tensor.transpose(pa[:], hid_nat[0:BPC, ts(c, 128)], ident[0:BPC, 0:BPC])
            nc.vector.tensor_copy(hidT[:, c, :], pa[:])

        # ---- Wk -> wkT[h(part), h-chunk c, o] (persistent), streamed per o-tile ----
        wkT = consts.tile([128, HT, H], F32R, tag="wkT")
        for t in range(HT):
            wnat = stage.tile([128, H], F32, tag="wnat")
            nc.sync.dma_start(wnat[:], wk_d[ts(t, 128), :])
            for c in range(HT):
                blk = tp.tile([128, 128], F32, tag="tp")
                nc.tensor.transpose(blk[:], wnat[:, ts(c, 128)], ident[:])
                nc.vector.tensor_copy(wkT[:, c, ts(t, 128)], blk[:])

        # ---- q^T + bq + bk: qkb[o(part), o-chunk t, b] ----
        # q^T[o_t] = sum_c Wq[o_t, h_c] @ hidT[h_c, :]; WqT blocks are transposed
        # on the fly and discarded.
        qkb = consts.tile([128, HT, BPC], F32, tag="qkb")
        for t in range(HT):
            wnat = stage.tile([128, H], F32, tag="wnat")
            nc.sync.dma_start(wnat[:], wq_d[ts(t, 128), :])
            pq = kp.tile([128, BPC], F32, tag="kp")
            for c in range(HT):
                blk = tp.tile([128, 128], F32, tag="tp")
                nc.tensor.transpose(blk[:], wnat[:, ts(c, 128)], ident[:])
                blks = stage.tile([128, 128], F32, tag="blks")
                nc.vector.tensor_copy(blks[:], blk[:])
                nc.tensor.matmul(
                    pq[:], blks[:], hidT[:, c, :], start=(c == 0), stop=(c == HT - 1)
                )
            nc.vector.tensor_scalar_add(qkb[:, t, :], pq[:], bsum[:, t : t + 1])

        # ---- main loop over the 4 local batches ----
        for b in range(BPC):
            enc_nat = encnat.tile([128, ST, H], F32R, tag="encnat")
            for t in range(ST):
                nc.sync.dma_start(enc_nat[:, t, :], enc_d[b, ts(t, 128), :])

            scores = batch.tile([1, S], F32, tag="scores")
            for j in range(SC):
                # transpose enc s-chunk j: encT_j[h(part), h-chunk c, s(512)]
                encT_j = encT_p.tile([128, HT, 512], F32R, tag="encTj")
                for c in range(HT):
                    for u in range(4):
                        blk = tp.tile([128, 128], F32, tag="tp")
                        nc.tensor.transpose(
                            blk[:],
                            enc_nat[:, 4 * j + u, ts(c, 128)].bitcast(F32),
                            ident[:],
                        )
                        nc.vector.tensor_copy(encT_j[:, c, ts(u, 128)], blk[:])

                # K^T tiles + fused bias/tanh -> eT_j[o(part), o-chunk i, s(512)]
                eT_j = eT_p.tile([128, HT, 512], F32R, tag="eTj")
                for i in range(HT):
                    pk = kp.tile([128, 512], F32, tag="kp")
                    for c in range(HT):
                        nc.tensor.matmul(
                            pk[:],
                            wkT[:, c, ts(i, 128)],
                            encT_j[:, c, :],
                            start=(c == 0),
                            stop=(c == HT - 1),
                        )
                    nc.scalar.activation(
                        eT_j[:, i, :], pk[:], Tanh, bias=qkb[:, i, b : b + 1]
                    )

                # scores chunk j = wv . eT_j  (contraction over o via PE)
                ps = vp.tile([1, 512], F32, tag="vp")
                for i in range(HT):
                    nc.tensor.matmul(
                        ps[:],
                        wvT[:, i : i + 1],
                        eT_j[:, i, :],
                        start=(i == 0),
                        stop=(i == HT - 1),
                    )
                nc.vector.tensor_copy(scores[0:1, ts(j, 512)], ps[:])

            # softmax over s (free dim, single partition)
            mx = batch.tile([1, 1], F32, tag="mx")
            nc.vector.reduce_max(mx[:], scores[:], axis=X)
            nmx = batch.tile([1, 1], F32, tag="nmx")
            nc.vector.tensor_scalar_mul(nmx[:], mx[:], -1.0)
            exps = batch.tile([1, S], F32, tag="exps")
            ssum = batch.tile([1, 1], F32, tag="ssum")
            nc.scalar.activation(
                exps[:], scores[:], Exp, bias=nmx[0:1, 0:1], accum_out=ssum[:]
            )
            inv = batch.tile([1, 1], F32, tag="inv")
            nc.vector.reciprocal(inv[:], ssum[:])

            # attn^T columns [s(part), s-tile]
            atT = batch.tile([128, ST], F32R, tag="atT")
            for t in range(ST):
                pa = tp.tile([128, 1], F32, tag="tp")
                nc.tensor.transpose(pa[:], exps[0:1, ts(t, 128)], ident[0:1, 0:1])
                nc.vector.tensor_copy(atT[:, t : t + 1], pa[:])

            # out[b] = (exp_scores @ enc) * inv_sum
            outb = batch.tile([1, H], F32, tag="outb")
            for hc in range(2):
                po = vp.tile([1, 512], F32, tag="vp")
                for t in range(ST):
                    nc.tensor.matmul(
                        po[:],
                        atT[:, t : t + 1],
                        enc_nat[:, t, ts(hc, 512)],
                        start=(t == 0),
                        stop=(t == ST - 1),
                    )
                nc.vector.tensor_scalar_mul(outb[0:1, ts(hc, 512)], po[:], inv[0:1, 0:1])
            nc.sync.dma_start(out_d[b], outb[:])

    nc.compile()
    return nc


_CACHED_NC = None


def _get_nc():
    global _CACHED_NC
    if _CACHED_NC is None:
        _CACHED_NC = build_program()
    return _CACHED_NC


def make_in_maps(hidden, encoder_outputs, Wq, bq, Wk, bk, wv):
    hid_last = np.ascontiguousarray(np.asarray(hidden, np.float32)[-1])  # [32, H]
    enc = np.asarray(encoder_outputs, np.float32)
    Wq = np.asarray(Wq, np.float32)
    Wk = np.asarray(Wk, np.float32)
    bq = np.asarray(bq, np.float32).reshape(1, H)
    bk = np.asarray(bk, np.float32).reshape(1, H)
    wv = np.asarray(wv, np.float32).reshape(1, H)
    in_maps = []
    for c in range(NCORES):
        sl = slice(c * BPC, (c + 1) * BPC)
        in_maps.append(
            {
                "hid": np.ascontiguousarray(hid_last[sl]),
                "enc": np.ascontiguousarray(enc[sl]),
                "wq": Wq,
                "wk": Wk,
                "bq": bq,
                "bk": bk,
                "wv": wv,
            }
        )
    return in_maps


def run(inputs, trace=False):
    """Run on hardware; returns (output [32,1,1024], BassKernelResults)."""
    nc = _get_nc()
    in_maps = make_in_maps(
        inputs["hidden"],
        inputs["encoder_outputs"],
        inputs["Wq"],
        inputs["bq"],
        inputs["Wk"],
        inputs["bk"],
        inputs["wv"],
    )
    res = run_bass_kernel_spmd(nc, in_maps, list(range(NCORES)), trace=trace)
    out = np.concatenate([res.results[c]["out"] for c in range(NCORES)], axis=0)
    return out.reshape(B, 1, H).astype(np.float32), res


def kernel(hidden, encoder_outputs, Wq, bq, Wk, bk, wv, bv):
    out, _ = run(
        {
            "hidden": hidden,
            "encoder_outputs": encoder_outputs,
            "Wq": Wq,
            "bq": bq,
            "Wk": Wk,
            "bk": bk,
            "wv": wv,
        }
    )
    return out
